# revision 13
# baseline (speedup 1.0000x reference)
"""Llama4 MoE layer on 8 Trainium2 NeuronCores — expert-parallel routed path,
tensor-parallel shared expert.

Sharding: the router runs on the host while sharding inputs. Core c receives

  - the tokens routed to expert c (pre-scaled by sigmoid(max logit)), padded
    to C1 columns — expert-parallel, no collective (outputs live on disjoint
    token sets, host scatter-adds), plus
  - a 512-wide F-slice (s = c%4) of the SHARED expert applied to half of
    all tokens (block b = c//4, 1024 tokens): tensor-parallel over F with
    G=4. Each core emits a bf16 partial [1024, D]; the host sums the 4
    partials per token block. This cuts the replicated shared weights from
    25MB/core to 6.25MB/core: total HBM traffic ~42MB/core vs ~55MB, so DMA
    (~300GB/s/core achieved) stays ahead of the PE instead of starving it.

Device kernel per core (identical SPMD program), three PE phases:
  A: shared gate/up (4 f-tiles x 2 512-col chunks) -> h_s
  B: shared down-proj units INTERLEAVED with expert gate/up units — the
     down units consume no new weights, so without interleaving the DMA
     sits idle behind a full gu ring during A->down, then the expert
     gate/up starves; interleaving keeps ring turnover (and HBM) busy.
  C: expert down-proj (16-chains over f, C1 cols).
Weight tiles stream as lhsT (stationary), tokens moving; silu*up in f32
PSUM -> h bf16 -> down tiles stationary, h streams; y bf16 half-strips DMA
out via the scalar engine's trigger queue (compute-dependent output DMAs
must not stall the sync engine's in-order weight prefetch stream; gpsimd
triggers are far too slow). Head: first gate/up/x tiles split into 4-dblock
pieces with triggers split across sync+scalar so the first chain starts
~1MB into the stream; a short warm-up drives the HAM clock gate while they
land. PE floor ~= 768 cyc/token * 528 token-equiv ~ 405K cycles ~ 169us.
"""

import sys

sys.path.insert(0, "/opt/trn_rl_repo")

import ml_dtypes
import numpy as np

import concourse.tile as tile
from concourse import bacc, mybir

T, D, F, E = 2048, 2048, 2048, 8
N_CORES = 8
P = 128
ND, NF = D // P, F // P
G = 4  # F-slices of the shared expert
B = N_CORES // G  # token blocks
C2 = T // B  # shared-expert tokens per core (1024)
NFS = NF // G  # shared f-tiles per core (4)
FS = F // G  # shared F columns per core (512)
QS = 512  # token chunk (PSUM bank width in f32)
DG = 4  # d-blocks per head tile piece
f32 = mybir.dt.float32
bf16 = mybir.dt.bfloat16


def build(C1):
    nc = bacc.Bacc(None, target_bir_lowering=False, debug=False)
    # x for shared wset, chunk-major: chunk q holds [P, ND*QS], d-major inside
    xsa = nc.declare_dram_parameter("xsa", [P, ND * C2], bf16, isOutput=False)
    xea = nc.declare_dram_parameter("xea", [P, ND * C1], bf16, isOutput=False)
    wgu = nc.declare_dram_parameter("wgu", [NF, P, 2 * ND * P], bf16, isOutput=False)
    wdp = nc.declare_dram_parameter(
        "wdp", [ND // 2, P, 2 * NF * P], bf16, isOutput=False
    )
    sgu = nc.declare_dram_parameter("sgu", [NFS, P, 2 * ND * P], bf16, isOutput=False)
    sdp = nc.declare_dram_parameter(
        "sdp", [ND // 2, P, 2 * NFS * P], bf16, isOutput=False
    )
    ye = nc.declare_dram_parameter("ye", [P, ND * C1], bf16, isOutput=True)
    ys = nc.declare_dram_parameter("ys", [P, ND * C2], bf16, isOutput=True)

    NPC = ND // DG  # head pieces (4)
    with tile.TileContext(nc) as tc:
        with (
            tc.tile_pool(name="xpool", bufs=1) as xp,
            tc.tile_pool(name="wstream", bufs=5) as wp,
            tc.tile_pool(name="hpool", bufs=1) as hp,
            tc.tile_pool(name="work", bufs=2) as sp,
            tc.tile_pool(name="ypool", bufs=3) as yp,
            tc.tile_pool(name="psGU", bufs=2, space="PSUM") as ppG,
            tc.tile_pool(name="psD", bufs=2, space="PSUM") as ppD,
            tc.tile_pool(name="psW", bufs=2, space="PSUM") as ppW,
        ):
            # ---- head: x chunk 0 / first gate / first up in 4-dblock pieces,
            # triggers alternating over the two hardware trigger engines
            # (sync + scalar) so the first gate chain starts ~1MB in ----
            xq0p, g0p, u0p = [], [], []
            for i in range(NPC):
                xt = xp.tile([P, DG * QS], bf16, tag=f"xq0_{i}", name=f"xq0_{i}")
                nc.sync.dma_start(
                    out=xt[:], in_=xsa[:, DG * QS * i : DG * QS * (i + 1)]
                )
                gt = wp.tile([P, DG * P], bf16, tag=f"g0_{i}", bufs=1, name=f"g0_{i}")
                nc.scalar.dma_start(
                    out=gt[:], in_=sgu[0, :, DG * P * i : DG * P * (i + 1)]
                )
                xq0p.append(xt)
                g0p.append(gt)
            # x chunk 1 next on sync (needed at ~14us, right after chunk 0);
            # all up-pieces go via scalar so they don't delay it
            xq1 = xp.tile([P, ND * QS], bf16, tag="xq1", name="xq1")
            nc.sync.dma_start(out=xq1[:], in_=xsa[:, ND * QS :])
            for i in range(NPC):
                ut = wp.tile([P, DG * P], bf16, tag=f"u0_{i}", bufs=1, name=f"u0_{i}")
                nc.scalar.dma_start(
                    out=ut[:], in_=sgu[0, :, ND * P + DG * P * i : ND * P + DG * P * (i + 1)]
                )
                u0p.append(ut)
            xe_t = xp.tile([P, ND * C1], bf16, tag="xea", name="xe_t")

            # HAM pre-warm: dummy PE activity while the head DMAs land
            warm = xp.tile([P, 256], bf16, tag="warm", name="warm")
            nc.vector.memset(warm[:], 0.0)
            for _ in range(12):
                wps = ppW.tile(
                    [P, 256], f32, space="PSUM", tag="warm", bufs=2, name="wps"
                )
                nc.tensor.matmul(
                    out=wps[:], lhsT=warm[:, :P], rhs=warm[:], start=True, stop=True
                )

            def xcol_shared(d, q0, qw):
                if q0 == 0:
                    t = xq0p[d // DG]
                    off = (d % DG) * QS
                else:
                    t, off = xq1, d * QS
                return t[:, off : off + qw]

            def gup_slice(f, d, up):
                # lhsT [P, 128] slice for shared f-tile f (f==0 from pieces)
                if f == 0:
                    t = (u0p if up else g0p)[d // DG]
                    return t[:, P * (d % DG) : P * (d % DG + 1)]
                t = sgu_tiles[f]
                off = ND * P if up else 0
                return t[:, off + P * d : off + P * (d + 1)]

            schunks = [(0, QS), (QS, QS)]
            echunks = []
            q0 = 0
            while q0 < C1:
                qw = min(QS, C1 - q0)
                echunks.append((q0, qw))
                q0 += qw

            def gu_phase(f, C, chunks, xcol, gsl, usl, tag):
                """gate/up chains for one f-tile -> h tile [P, C] bf16"""
                h_t = hp.tile([P, C], bf16, tag=tag, bufs=1, name=tag)
                for q0, qw in chunks:
                    pg = ppG.tile([P, qw], f32, space="PSUM", tag="pg", name="pg")
                    pu = ppG.tile([P, qw], f32, space="PSUM", tag="pu", name="pu")
                    for d in range(ND):
                        nc.tensor.matmul(
                            out=pg[:], lhsT=gsl(d), rhs=xcol(d, q0, qw),
                            start=(d == 0), stop=(d == ND - 1),
                        )
                    for d in range(ND):
                        nc.tensor.matmul(
                            out=pu[:], lhsT=usl(d), rhs=xcol(d, q0, qw),
                            start=(d == 0), stop=(d == ND - 1),
                        )
                    sig = sp.tile([P, qw], f32, tag="sig", name="sig")
                    nc.scalar.activation(
                        sig[:], pg[:], mybir.ActivationFunctionType.Sigmoid
                    )
                    nc.vector.tensor_tensor(
                        out=sig[:], in0=sig[:], in1=pg[:], op=mybir.AluOpType.mult
                    )
                    nc.vector.tensor_tensor(
                        out=h_t[:, q0 : q0 + qw], in0=sig[:], in1=pu[:],
                        op=mybir.AluOpType.mult,
                    )
                return h_t

            def down_half(dt_, nf, half, j, C, chunks, h_tiles, y_p, ytag):
                """one (j, half) down unit -> y half-strip DMA out via scalar"""
                y_t = down_half.y
                if half == 0:
                    y_t = down_half.y = yp.tile(
                        [P, 2 * C], bf16, tag=ytag, name=f"{ytag}_{j}"
                    )
                for q0, qw in chunks:
                    py = ppD.tile([P, qw], f32, space="PSUM", tag="py", name="py")
                    for f in range(nf):
                        nc.tensor.matmul(
                            out=py[:], lhsT=dt_[:, P * f : P * (f + 1)],
                            rhs=h_tiles[f][:, q0 : q0 + qw],
                            start=(f == 0), stop=(f == nf - 1),
                        )
                    nc.vector.tensor_copy(
                        y_t[:, C * half + q0 : C * half + q0 + qw], py[:]
                    )
                nc.scalar.dma_start(
                    out=y_p[:, C * (2 * j + half) : C * (2 * j + half + 1)],
                    in_=y_t[:, C * half : C * (half + 1)],
                )

            down_half.y = None

            # ---- phase A: shared gate/up ----
            sgu_tiles = {}
            hs = []
            for f in range(NFS):
                if f > 0:
                    gu = wp.tile([P, 2 * ND * P], bf16, tag="wgu", name=f"sgu{f}")
                    nc.sync.dma_start(out=gu[:], in_=sgu[f])
                    sgu_tiles[f] = gu
                if f == 2:
                    # expert-token x, deferred past the startup crunch
                    nc.sync.dma_start(out=xe_t[:], in_=xea[:])
                hs.append(
                    gu_phase(
                        f, C2, schunks, xcol_shared,
                        lambda d, f=f: gup_slice(f, d, False),
                        lambda d, f=f: gup_slice(f, d, True),
                        f"hs{f}",
                    )
                )

            # shared down tiles: full ring depth so the in-order sync engine
            # fires all of them upfront and flows on to expert weights
            sdd = []
            for j in range(ND // 2):
                dd = wp.tile([P, 2 * NFS * P], bf16, tag="wd0", bufs=8, name=f"sdd{j}")
                nc.sync.dma_start(out=dd[:], in_=sdp[j])
                sdd.append(dd)

            # ---- phase B: shared down units interleaved with expert gate/up ----
            he = []

            def eg_unit(f):
                gu = wp.tile([P, 2 * ND * P], bf16, tag="wgu", name=f"egu{f}")
                nc.sync.dma_start(out=gu[:], in_=wgu[f])
                he.append(
                    gu_phase(
                        f, C1, echunks,
                        lambda d, q0, qw: xe_t[:, C1 * d + q0 : C1 * d + q0 + qw],
                        lambda d: gu[:, P * d : P * (d + 1)],
                        lambda d: gu[:, ND * P + P * d : ND * P + P * (d + 1)],
                        f"he{f}",
                    )
                )

            # expert down tiles in 0.5MB d-block halves (ring 10) so the DMA
            # idle window late in phase B prefills the expert-down stream;
            # triggers for the first ten interleave into the tail of phase B
            edd = {}

            def edd_fire(jh):
                dblk, hh = jh // 2, jh % 2
                t = wp.tile([P, NF * P], bf16, tag="wd1", bufs=10, name=f"edd{jh}")
                nc.sync.dma_start(
                    out=t[:], in_=wdp[dblk, :, NF * P * hh : NF * P * (hh + 1)]
                )
                edd[jh] = t

            for k in range(16):
                j, half = k // 2, k % 2
                down_half(
                    sdd[j][:, NFS * P * half : NFS * P * (half + 1)],
                    NFS, half, j, C2, schunks, hs, ys, "y0",
                )
                eg_unit(k)
                if k >= 11:
                    edd_fire(2 * (k - 11))
                    edd_fire(2 * (k - 11) + 1)

            # ---- phase C: expert down ----
            for jh in range(ND):
                if jh not in edd:
                    edd_fire(jh)
                down_half(
                    edd[jh][:], NF, jh % 2, jh // 2, C1, echunks, he, ye, "y1"
                )
    nc.finalize()
    return nc


def _tile_lhsT(w):
    # [A, B] f32 -> [B/P, P, A] bf16 : block b, partition p(a%P), col a_blk*P+q
    A, B = w.shape
    return np.ascontiguousarray(
        w.reshape(A // P, P, B // P, P).transpose(2, 1, 0, 3).reshape(B // P, P, A)
    ).astype(ml_dtypes.bfloat16)


def _fuse_gu(g, u):
    return np.ascontiguousarray(
        np.concatenate([_tile_lhsT(g), _tile_lhsT(u)], axis=2)
    )


def _fuse_dpairs(dw):
    t = _tile_lhsT(dw)
    return np.ascontiguousarray(np.concatenate([t[0::2], t[1::2]], axis=2))


def _pack_x(xc):
    # [C, D] f32 -> [P, ND*C] bf16 with row p holding all d-blocks' row p
    C = xc.shape[0]
    return np.ascontiguousarray(
        xc.T.reshape(ND, P, C).transpose(1, 0, 2).reshape(P, ND * C)
    ).astype(ml_dtypes.bfloat16)


def _pack_x_chunked(xc, qs=QS):
    # chunk-major: concat per-chunk _pack_x along cols
    return np.ascontiguousarray(
        np.concatenate(
            [_pack_x(xc[q : q + qs]) for q in range(0, xc.shape[0], qs)], axis=1
        )
    )


def _unpack_y(ya, C):
    # [P, ND*C] bf16 -> [C, D] f32
    return (
        np.asarray(ya)
        .reshape(P, ND, C)
        .transpose(2, 1, 0)
        .reshape(C, D)
        .astype(np.float32)
    )


def _prep(inputs):
    x = np.asarray(inputs["hidden_states"], dtype=np.float32).reshape(T, D)
    rw = np.asarray(inputs["router_w"], np.float32)

    # router: top-1 expert + sigmoid(max logit) scale, computed while sharding
    logits = x @ rw
    eidx = logits.argmax(-1)
    score = 1.0 / (1.0 + np.exp(-logits.max(-1)))
    xs = x * score[:, None]

    idx = [np.nonzero(eidx == c)[0] for c in range(N_CORES)]
    maxn = max(len(i) for i in idx)
    C1 = max(16, -(-maxn // 16) * 16)

    sg = np.asarray(inputs["shared_gate_w"], np.float32)
    su = np.asarray(inputs["shared_up_w"], np.float32)
    sd = np.asarray(inputs["shared_down_w"], np.float32)
    gw_all = np.asarray(inputs["gate_w"], np.float32)
    uw_all = np.asarray(inputs["up_w"], np.float32)
    dw_all = np.asarray(inputs["down_w"], np.float32)

    in_maps = []
    for c in range(N_CORES):
        b, s = c // G, c % G
        xe = np.zeros((C1, D), np.float32)
        xe[: len(idx[c])] = xs[idx[c]]
        in_maps.append(
            {
                "xsa": _pack_x_chunked(x[C2 * b : C2 * (b + 1)]),
                "xea": _pack_x(xe),
                "wgu": _fuse_gu(gw_all[c], uw_all[c]),
                "wdp": _fuse_dpairs(dw_all[c]),
                "sgu": _fuse_gu(
                    sg[:, FS * s : FS * (s + 1)], su[:, FS * s : FS * (s + 1)]
                ),
                "sdp": _fuse_dpairs(sd[FS * s : FS * (s + 1)]),
            }
        )
    return in_maps, idx, C1


def run(inputs, trace=False, tmpdir=None):
    from concourse.bass_utils import run_bass_kernel_spmd

    in_maps, idx, C1 = _prep(inputs)
    nc = build(C1)
    res = run_bass_kernel_spmd(
        nc, in_maps, core_ids=list(range(N_CORES)), trace=trace, tmpdir=tmpdir
    )
    out = np.zeros((T, D), np.float32)
    for c in range(N_CORES):
        b = c // G
        ye = _unpack_y(res.results[c]["ye"], C1)
        out[idx[c]] += ye[: len(idx[c])]
        out[C2 * b : C2 * (b + 1)] += _unpack_y(res.results[c]["ys"], C2)
    return out.reshape(T // 2, 2, D), res


def kernel(**inputs) -> np.ndarray:
    out, _ = run(inputs)
    return out


# revision 15
# speedup vs baseline: 1.0400x; 1.0400x over previous
"""Llama4 MoE layer on 8 Trainium2 NeuronCores — expert-parallel routed path,
tensor-parallel shared expert.

Sharding: the router runs on the host while sharding inputs. Core c receives

  - the tokens routed to expert c (pre-scaled by sigmoid(max logit)), padded
    to C1 columns — expert-parallel, no collective (outputs live on disjoint
    token sets, host scatter-adds), plus
  - a 512-wide F-slice (s = c%4) of the SHARED expert applied to half of
    all tokens (block b = c//4, 1024 tokens): tensor-parallel over F with
    G=4. Each core emits a bf16 partial [1024, D]; the host sums the 4
    partials per token block. This cuts the replicated shared weights from
    25MB/core to 6.25MB/core: total HBM traffic ~42MB/core vs ~55MB, so DMA
    (~300GB/s/core achieved) stays ahead of the PE instead of starving it.

Device kernel per core (identical SPMD program), three PE phases:
  A: shared gate/up (4 f-tiles x 2 512-col chunks) -> h_s
  B: shared down-proj units INTERLEAVED with expert gate/up units — the
     down units consume no new weights, so without interleaving the DMA
     sits idle behind a full gu ring during A->down, then the expert
     gate/up starves; interleaving keeps ring turnover (and HBM) busy.
  C: expert down-proj (16-chains over f, C1 cols).
Weight tiles stream as lhsT (stationary), tokens moving; silu*up in f32
PSUM -> h bf16 -> down tiles stationary, h streams; y bf16 half-strips DMA
out via the scalar engine's trigger queue (compute-dependent output DMAs
must not stall the sync engine's in-order weight prefetch stream; gpsimd
triggers are far too slow). Head: first gate/up/x tiles split into 4-dblock
pieces with triggers split across sync+scalar so the first chain starts
~1MB into the stream; a short warm-up drives the HAM clock gate while they
land. PE floor ~= 768 cyc/token * 528 token-equiv ~ 405K cycles ~ 169us.
"""

import sys

sys.path.insert(0, "/opt/trn_rl_repo")

import ml_dtypes
import numpy as np

import concourse.tile as tile
from concourse import bacc, mybir

T, D, F, E = 2048, 2048, 2048, 8
N_CORES = 8
P = 128
ND, NF = D // P, F // P
G = 4  # F-slices of the shared expert
B = N_CORES // G  # token blocks
C2 = T // B  # shared-expert tokens per core (1024)
NFS = NF // G  # shared f-tiles per core (4)
FS = F // G  # shared F columns per core (512)
QS = 512  # token chunk (PSUM bank width in f32)
DG = 4  # d-blocks per head tile piece
f32 = mybir.dt.float32
bf16 = mybir.dt.bfloat16


def build(C1):
    nc = bacc.Bacc(None, target_bir_lowering=False, debug=False)
    # x for shared wset, chunk-major: chunk q holds [P, ND*QS], d-major inside
    xsa = nc.declare_dram_parameter("xsa", [P, ND * C2], bf16, isOutput=False)
    xea = nc.declare_dram_parameter("xea", [P, ND * C1], bf16, isOutput=False)
    wgu = nc.declare_dram_parameter("wgu", [NF, P, 2 * ND * P], bf16, isOutput=False)
    wdp = nc.declare_dram_parameter(
        "wdp", [ND // 2, P, 2 * NF * P], bf16, isOutput=False
    )
    sgu = nc.declare_dram_parameter("sgu", [NFS, P, 2 * ND * P], bf16, isOutput=False)
    sdp = nc.declare_dram_parameter(
        "sdp", [ND // 2, P, 2 * NFS * P], bf16, isOutput=False
    )
    ye = nc.declare_dram_parameter("ye", [P, ND * C1], bf16, isOutput=True)
    ys = nc.declare_dram_parameter("ys", [P, ND * C2], bf16, isOutput=True)

    NPC = ND // DG  # head pieces (4)
    with tile.TileContext(nc) as tc:
        with (
            tc.tile_pool(name="xpool", bufs=1) as xp,
            tc.tile_pool(name="wstream", bufs=5) as wp,
            tc.tile_pool(name="hpool", bufs=1) as hp,
            tc.tile_pool(name="work", bufs=2) as sp,
            tc.tile_pool(name="ypool", bufs=3) as yp,
            tc.tile_pool(name="psGU", bufs=2, space="PSUM") as ppG,
            tc.tile_pool(name="psD", bufs=2, space="PSUM") as ppD,
            tc.tile_pool(name="psW", bufs=2, space="PSUM") as ppW,
        ):
            # ---- head: x chunk 0 / first gate / first up in 4-dblock pieces,
            # triggers alternating over the two hardware trigger engines
            # (sync + scalar) so the first gate chain starts ~1MB in ----
            xq0p, g0p, u0p = [], [], []
            for i in range(NPC):
                xt = xp.tile([P, DG * QS], bf16, tag=f"xq0_{i}", name=f"xq0_{i}")
                nc.sync.dma_start(
                    out=xt[:], in_=xsa[:, DG * QS * i : DG * QS * (i + 1)]
                )
                gt = wp.tile([P, DG * P], bf16, tag=f"g0_{i}", bufs=1, name=f"g0_{i}")
                nc.scalar.dma_start(
                    out=gt[:], in_=sgu[0, :, DG * P * i : DG * P * (i + 1)]
                )
                xq0p.append(xt)
                g0p.append(gt)
            for i in range(NPC):
                ut = wp.tile([P, DG * P], bf16, tag=f"u0_{i}", bufs=1, name=f"u0_{i}")
                (nc.scalar if i % 2 else nc.sync).dma_start(
                    out=ut[:], in_=sgu[0, :, ND * P + DG * P * i : ND * P + DG * P * (i + 1)]
                )
                u0p.append(ut)
            xq1 = xp.tile([P, ND * QS], bf16, tag="xq1", name="xq1")
            nc.sync.dma_start(out=xq1[:], in_=xsa[:, ND * QS :])
            xe_t = xp.tile([P, ND * C1], bf16, tag="xea", name="xe_t")

            # HAM pre-warm: dummy PE activity while the head DMAs land
            warm = xp.tile([P, 256], bf16, tag="warm", name="warm")
            nc.vector.memset(warm[:], 0.0)
            for _ in range(12):
                wps = ppW.tile(
                    [P, 256], f32, space="PSUM", tag="warm", bufs=2, name="wps"
                )
                nc.tensor.matmul(
                    out=wps[:], lhsT=warm[:, :P], rhs=warm[:], start=True, stop=True
                )

            def xcol_shared(d, q0, qw):
                if q0 == 0:
                    t = xq0p[d // DG]
                    off = (d % DG) * QS
                else:
                    t, off = xq1, d * QS
                return t[:, off : off + qw]

            def gup_slice(f, d, up):
                # lhsT [P, 128] slice for shared f-tile f (f==0 from pieces)
                if f == 0:
                    t = (u0p if up else g0p)[d // DG]
                    return t[:, P * (d % DG) : P * (d % DG + 1)]
                t = sgu_tiles[f]
                off = ND * P if up else 0
                return t[:, off + P * d : off + P * (d + 1)]

            schunks = [(0, QS), (QS, QS)]
            echunks = []
            q0 = 0
            while q0 < C1:
                qw = min(QS, C1 - q0)
                echunks.append((q0, qw))
                q0 += qw

            def gu_phase(f, C, chunks, xcol, gsl, usl, tag):
                """gate/up chains for one f-tile -> h tile [P, C] bf16"""
                h_t = hp.tile([P, C], bf16, tag=tag, bufs=1, name=tag)
                for q0, qw in chunks:
                    pg = ppG.tile([P, qw], f32, space="PSUM", tag="pg", name="pg")
                    pu = ppG.tile([P, qw], f32, space="PSUM", tag="pu", name="pu")
                    for d in range(ND):
                        nc.tensor.matmul(
                            out=pg[:], lhsT=gsl(d), rhs=xcol(d, q0, qw),
                            start=(d == 0), stop=(d == ND - 1),
                        )
                    for d in range(ND):
                        nc.tensor.matmul(
                            out=pu[:], lhsT=usl(d), rhs=xcol(d, q0, qw),
                            start=(d == 0), stop=(d == ND - 1),
                        )
                    sig = sp.tile([P, qw], f32, tag="sig", name="sig")
                    nc.scalar.activation(
                        sig[:], pg[:], mybir.ActivationFunctionType.Sigmoid
                    )
                    nc.vector.tensor_tensor(
                        out=sig[:], in0=sig[:], in1=pg[:], op=mybir.AluOpType.mult
                    )
                    nc.vector.tensor_tensor(
                        out=h_t[:, q0 : q0 + qw], in0=sig[:], in1=pu[:],
                        op=mybir.AluOpType.mult,
                    )
                return h_t

            def down_half(dt_, nf, half, j, C, chunks, h_tiles, y_p, ytag):
                """one (j, half) down unit -> y half-strip DMA out via scalar"""
                y_t = down_half.y
                if half == 0:
                    y_t = down_half.y = yp.tile(
                        [P, 2 * C], bf16, tag=ytag, name=f"{ytag}_{j}"
                    )
                for q0, qw in chunks:
                    py = ppD.tile([P, qw], f32, space="PSUM", tag="py", name="py")
                    for f in range(nf):
                        nc.tensor.matmul(
                            out=py[:], lhsT=dt_[:, P * f : P * (f + 1)],
                            rhs=h_tiles[f][:, q0 : q0 + qw],
                            start=(f == 0), stop=(f == nf - 1),
                        )
                    nc.vector.tensor_copy(
                        y_t[:, C * half + q0 : C * half + q0 + qw], py[:]
                    )
                # shared strips go out per full d-pair (fewer, bigger HBM
                # writes -> fewer read/write turnarounds mid-kernel); expert
                # strips per half so the final strip drains fast at the tail
                if ytag == "y0":
                    if half == 1:
                        nc.scalar.dma_start(
                            out=y_p[:, C * 2 * j : C * 2 * (j + 1)], in_=y_t[:]
                        )
                else:
                    nc.scalar.dma_start(
                        out=y_p[:, C * (2 * j + half) : C * (2 * j + half + 1)],
                        in_=y_t[:, C * half : C * (half + 1)],
                    )

            down_half.y = None

            # ---- phase A: shared gate/up ----
            sgu_tiles = {}
            hs = []
            for f in range(NFS):
                if f > 0:
                    gu = wp.tile([P, 2 * ND * P], bf16, tag="wgu", name=f"sgu{f}")
                    nc.sync.dma_start(out=gu[:], in_=sgu[f])
                    sgu_tiles[f] = gu
                if f == 2:
                    # expert-token x, deferred past the startup crunch
                    nc.sync.dma_start(out=xe_t[:], in_=xea[:])
                hs.append(
                    gu_phase(
                        f, C2, schunks, xcol_shared,
                        lambda d, f=f: gup_slice(f, d, False),
                        lambda d, f=f: gup_slice(f, d, True),
                        f"hs{f}",
                    )
                )

            # shared down tiles: full ring depth so the in-order sync engine
            # fires all of them upfront and flows on to expert weights
            sdd = []
            for j in range(ND // 2):
                dd = wp.tile([P, 2 * NFS * P], bf16, tag="wd0", bufs=8, name=f"sdd{j}")
                nc.sync.dma_start(out=dd[:], in_=sdp[j])
                sdd.append(dd)

            # ---- phase B: shared down units interleaved with expert gate/up ----
            he = []

            def eg_unit(f):
                gu = wp.tile([P, 2 * ND * P], bf16, tag="wgu", name=f"egu{f}")
                nc.sync.dma_start(out=gu[:], in_=wgu[f])
                he.append(
                    gu_phase(
                        f, C1, echunks,
                        lambda d, q0, qw: xe_t[:, C1 * d + q0 : C1 * d + q0 + qw],
                        lambda d: gu[:, P * d : P * (d + 1)],
                        lambda d: gu[:, ND * P + P * d : ND * P + P * (d + 1)],
                        f"he{f}",
                    )
                )

            # expert down tiles in 0.5MB d-block halves (ring 10) so the DMA
            # idle window late in phase B prefills the expert-down stream;
            # triggers for the first ten interleave into the tail of phase B
            edd = {}

            def edd_fire(jh):
                dblk, hh = jh // 2, jh % 2
                t = wp.tile([P, NF * P], bf16, tag="wd1", bufs=10, name=f"edd{jh}")
                nc.sync.dma_start(
                    out=t[:], in_=wdp[dblk, :, NF * P * hh : NF * P * (hh + 1)]
                )
                edd[jh] = t

            for k in range(16):
                j, half = k // 2, k % 2
                down_half(
                    sdd[j][:, NFS * P * half : NFS * P * (half + 1)],
                    NFS, half, j, C2, schunks, hs, ys, "y0",
                )
                eg_unit(k)
                if k >= 11:
                    edd_fire(2 * (k - 11))
                    edd_fire(2 * (k - 11) + 1)

            # ---- phase C: expert down ----
            for jh in range(ND):
                if jh not in edd:
                    edd_fire(jh)
                down_half(
                    edd[jh][:], NF, jh % 2, jh // 2, C1, echunks, he, ye, "y1"
                )
    nc.finalize()
    return nc


def _tile_lhsT(w):
    # [A, B] f32 -> [B/P, P, A] bf16 : block b, partition p(a%P), col a_blk*P+q
    A, B = w.shape
    return np.ascontiguousarray(
        w.reshape(A // P, P, B // P, P).transpose(2, 1, 0, 3).reshape(B // P, P, A)
    ).astype(ml_dtypes.bfloat16)


def _fuse_gu(g, u):
    return np.ascontiguousarray(
        np.concatenate([_tile_lhsT(g), _tile_lhsT(u)], axis=2)
    )


def _fuse_dpairs(dw):
    t = _tile_lhsT(dw)
    return np.ascontiguousarray(np.concatenate([t[0::2], t[1::2]], axis=2))


def _pack_x(xc):
    # [C, D] f32 -> [P, ND*C] bf16 with row p holding all d-blocks' row p
    C = xc.shape[0]
    return np.ascontiguousarray(
        xc.T.reshape(ND, P, C).transpose(1, 0, 2).reshape(P, ND * C)
    ).astype(ml_dtypes.bfloat16)


def _pack_x_chunked(xc, qs=QS):
    # chunk-major: concat per-chunk _pack_x along cols
    return np.ascontiguousarray(
        np.concatenate(
            [_pack_x(xc[q : q + qs]) for q in range(0, xc.shape[0], qs)], axis=1
        )
    )


def _unpack_y(ya, C):
    # [P, ND*C] bf16 -> [C, D] f32
    return (
        np.asarray(ya)
        .reshape(P, ND, C)
        .transpose(2, 1, 0)
        .reshape(C, D)
        .astype(np.float32)
    )


def _prep(inputs):
    x = np.asarray(inputs["hidden_states"], dtype=np.float32).reshape(T, D)
    rw = np.asarray(inputs["router_w"], np.float32)

    # router: top-1 expert + sigmoid(max logit) scale, computed while sharding
    logits = x @ rw
    eidx = logits.argmax(-1)
    score = 1.0 / (1.0 + np.exp(-logits.max(-1)))
    xs = x * score[:, None]

    idx = [np.nonzero(eidx == c)[0] for c in range(N_CORES)]
    maxn = max(len(i) for i in idx)
    C1 = max(16, -(-maxn // 16) * 16)

    sg = np.asarray(inputs["shared_gate_w"], np.float32)
    su = np.asarray(inputs["shared_up_w"], np.float32)
    sd = np.asarray(inputs["shared_down_w"], np.float32)
    gw_all = np.asarray(inputs["gate_w"], np.float32)
    uw_all = np.asarray(inputs["up_w"], np.float32)
    dw_all = np.asarray(inputs["down_w"], np.float32)

    in_maps = []
    for c in range(N_CORES):
        b, s = c // G, c % G
        xe = np.zeros((C1, D), np.float32)
        xe[: len(idx[c])] = xs[idx[c]]
        in_maps.append(
            {
                "xsa": _pack_x_chunked(x[C2 * b : C2 * (b + 1)]),
                "xea": _pack_x(xe),
                "wgu": _fuse_gu(gw_all[c], uw_all[c]),
                "wdp": _fuse_dpairs(dw_all[c]),
                "sgu": _fuse_gu(
                    sg[:, FS * s : FS * (s + 1)], su[:, FS * s : FS * (s + 1)]
                ),
                "sdp": _fuse_dpairs(sd[FS * s : FS * (s + 1)]),
            }
        )
    return in_maps, idx, C1


def run(inputs, trace=False, tmpdir=None):
    from concourse.bass_utils import run_bass_kernel_spmd

    in_maps, idx, C1 = _prep(inputs)
    nc = build(C1)
    res = run_bass_kernel_spmd(
        nc, in_maps, core_ids=list(range(N_CORES)), trace=trace, tmpdir=tmpdir
    )
    out = np.zeros((T, D), np.float32)
    for c in range(N_CORES):
        b = c // G
        ye = _unpack_y(res.results[c]["ye"], C1)
        out[idx[c]] += ye[: len(idx[c])]
        out[C2 * b : C2 * (b + 1)] += _unpack_y(res.results[c]["ys"], C2)
    return out.reshape(T // 2, 2, D), res


def kernel(**inputs) -> np.ndarray:
    out, _ = run(inputs)
    return out
